# revision 9
# baseline (speedup 1.0000x reference)
"""Trainium2 Bass kernel for causal cosine-sim multi-head attention.

Reference computation (per batch b of 4, 2048 tokens, dim 1024):
  q,k,v = x @ Wq, x @ Wk, x @ Wv          (inner = 8 heads x 64)
  q,k l2-normalized per head, scale 8.0, causal softmax, out = attn-out @ Wo

Sharding: 8 cores = 4 batches x 2 head-groups (4 heads each).
Core c handles batch c//2, heads [4*(c%2), 4*(c%2)+4).  Each core computes a
partial output (2048, 1024) = o_g @ Wo_g; host sums the two head-group
partials per batch.  No on-chip collectives; the 8 cores run SPMD.

v1 layout (vs baseline): QKV-projection, attention and output-projection are
interleaved per 512-token i-tile so the PE never idles long enough for HAM
re-throttle and the scalar-engine exp stream overlaps projection matmuls.
All partition-broadcast work (l2-norm scales, softmax denominators) moved
off the tensor engine onto gpsimd (partition_all_reduce / partition_
broadcast); softmax denominators use vector.reciprocal directly on the PSUM
ones-row.  Causal diagonal blocks compute only the needed column ranges in
S / exp / AV.
"""

import numpy as np

import concourse.bass as bass
import concourse.bacc as bacc
import concourse.bass_isa as bass_isa
import concourse.mybir as mybir
import concourse.tile as tile
from concourse.bass_utils import run_bass_kernel_spmd

DT = mybir.dt
F32 = DT.float32
BF16 = DT.bfloat16

N_TOK = 2048
DIM = 1024
DG = 256          # inner dims per core (4 heads x 64)
NH = 4            # heads per core
DH = 64
MOUT = 1024


def build_nc(N=N_TOK):
    NKC = DIM // 128          # 8 contraction chunks
    NTC = N // 128            # token chunks
    QT = 512                  # token tile (qkv projection and attention i)
    NQT = N // QT
    AF = mybir.ActivationFunctionType
    RED = bass_isa.ReduceOp

    nc = bacc.Bacc("TRN2", target_bir_lowering=False, debug=False, num_devices=8)
    xt_ext = nc.dram_tensor("xt", [DIM, N], BF16, kind="ExternalInput")
    wq_ext = nc.dram_tensor("wq", [128, NKC, DG], BF16, kind="ExternalInput")
    wk_ext = nc.dram_tensor("wk", [128, NKC, DG], BF16, kind="ExternalInput")
    wv_ext = nc.dram_tensor("wv", [128, NKC, DG], BF16, kind="ExternalInput")
    wo_ext = nc.dram_tensor("wo", [128, DG // 128, MOUT], BF16,
                            kind="ExternalInput")
    out_ext = nc.dram_tensor("out", [MOUT, N], BF16, kind="ExternalOutput")

    with tile.TileContext(nc) as tc:
        with (
            tc.tile_pool(name="persist", bufs=1) as pp,
            tc.tile_pool(name="stage", bufs=3) as st,
            tc.tile_pool(name="attn_sb", bufs=4) as asb,
            tc.tile_pool(name="ps_mm", bufs=2, space="PSUM") as psM,
            tc.tile_pool(name="ps_s", bufs=2, space="PSUM") as psS,
            tc.tile_pool(name="ps_o", bufs=2, space="PSUM") as psO,
        ):
            xt = pp.tile([128, NKC, N], BF16, tag="xt")          # x transposed
            wq_sb = pp.tile([128, NKC, DG], BF16, tag="wq")
            wk_sb = pp.tile([128, NKC, DG], BF16, tag="wk")
            wv_sb = pp.tile([128, NKC, DG], BF16, tag="wv")
            wo_sb = pp.tile([128, 2, MOUT], BF16, tag="wo")
            # per-head, base partition 0 (matmul inputs at base>=64 fault)
            qts = pp.tile([64, NH, N], BF16, tag="qts")          # scaled Q^T
            kts = pp.tile([64, NH, N], BF16, tag="kts")          # scaled K^T
            vt = pp.tile([128, NTC, NH, DH + 1], BF16, tag="vt")  # [V | 1]
            ot = pp.tile([128, 2, N], BF16, tag="ot")            # normed O^T
            # 1/softmax-sum, one tile per head: gpsimd partition_broadcast
            # only reads APs at partition offset 0
            inv = [pp.tile([1, N], F32, tag=f"inv{h}", name=f"inv{h}")
                   for h in range(NH)]
            # causal mask for the partial 128 columns of a diagonal block,
            # duplicated for the 2 heads of a pair: keep where col >= row
            mask2 = pp.tile([128, 2, 128], BF16, tag="mask2")
            ones2 = pp.tile([128, 2, 128], BF16, tag="ones2")
            nc.vector.memset(ones2[:, :, :], 1.0)
            nc.gpsimd.affine_select(
                mask2[:, :, :], ones2[:, :, :], pattern=[[0, 2], [1, 128]],
                compare_op=mybir.AluOpType.is_ge, fill=0.0,
                base=0, channel_multiplier=-1)

            # ---- input DMAs: 3 queues, first-tile data first, wo last ----
            xv = xt_ext.rearrange("(c p) n -> p c n", p=128)
            nc.sync.dma_start(wq_sb[:, :, :], wq_ext.ap())
            nc.scalar.dma_start(wv_sb[:, :, :], wv_ext.ap())
            nc.gpsimd.dma_start(wk_sb[:, :, :], wk_ext.ap())
            for ch in range(NQT):
                csl = slice(ch * QT, (ch + 1) * QT)
                nc.sync.dma_start(xt[:, 0:4, csl], xv[:, 0:4, csl])
                nc.scalar.dma_start(xt[:, 4:8, csl], xv[:, 4:8, csl])
            nc.gpsimd.dma_start(wo_sb[:, :, :], wo_ext.ap())

            def qkv_tile(t):
                tsl = slice(t * QT, (t + 1) * QT)
                for wsb, dst, sqscale in (
                    (wq_sb, qts, 1.0 / 64.0),   # arsqrt(nq/64) = 8/||q||
                    (wk_sb, kts, 1.0),          # 1/||k||
                ):
                    for dc in range(2):
                        pps = psM.tile([128, QT], F32, tag="mm_ps")
                        for kc in range(NKC):
                            nc.tensor.matmul(
                                pps[:, :],
                                wsb[:, kc, dc * 128:(dc + 1) * 128],
                                xt[:, kc, tsl],
                                start=(kc == 0), stop=(kc == NKC - 1))
                        # stage both psum halves at partition base 0: gpsimd
                        # ISA ops misread APs with partition offset != 0, and
                        # DVE inputs must share partitions
                        qsb = st.tile([64, 2, QT], F32, tag="qsb", bufs=4)
                        for half in range(2):
                            pr = 64 * half
                            nc.vector.tensor_copy(qsb[:, half, :],
                                                  pps[pr:pr + 64, :])
                        sq = st.tile([64, 2, QT], BF16, tag="sq", bufs=2)
                        n2 = st.tile([64, 2, QT], F32, tag="n2", bufs=2)
                        for half in range(2):
                            nc.vector.tensor_mul(sq[:, half, :],
                                                 qsb[:, half, :],
                                                 qsb[:, half, :])
                            nc.gpsimd.partition_all_reduce(
                                n2[:, half, :], sq[:, half, :],
                                channels=64, reduce_op=RED.add)
                        rs = st.tile([64, 2, QT], F32, tag="rs", bufs=2)
                        nc.scalar.activation(rs[:, :, :], n2[:, :, :],
                                             AF.Abs_reciprocal_sqrt,
                                             scale=sqscale)
                        for half in range(2):
                            nc.vector.tensor_mul(
                                dst[0:64, 2 * dc + half, tsl],
                                qsb[:, half, :], rs[:, half, :])
                # V for the 4 token-chunks of this tile, with ones column
                for tcc in range(4 * t, 4 * t + 4):
                    vps = psM.tile([128, QT], F32, tag="mm_ps")
                    for kc in range(NKC):
                        nc.tensor.matmul(
                            vps[:, 0:DG],
                            xt[:, kc, tcc * 128:(tcc + 1) * 128],
                            wv_sb[:, kc, :],
                            start=(kc == 0), stop=(kc == NKC - 1))
                    nc.vector.tensor_copy(
                        vt[:, tcc, :, 0:64],
                        vps[:, 0:DG].rearrange("p (h d) -> p h d", d=64))
                    nc.vector.memset(vt[:, tcc, :, 64:65], 1.0)

            def attn_tile(t):
                isl = slice(t * QT, (t + 1) * QT)
                njc = 4 * (t + 1)
                for p in range(2):          # head pair = (2p, 2p+1)
                    h0, h1 = 2 * p, 2 * p + 1
                    o_ps = [psO.tile([65, QT], F32, tag="o_ps",
                                     name=f"o_{t}_{p}_{hh}")
                            for hh in range(2)]
                    for jc in range(njc):
                        jsl = slice(jc * 128, (jc + 1) * 128)
                        off = 128 * (jc - 4 * t) if jc >= 4 * t else 0
                        s2 = psS.tile([128, 2, QT], F32, tag="s2")
                        for hh, h in ((0, h0), (1, h1)):
                            nc.tensor.matmul(
                                s2[:, hh, off:], kts[0:64, h, jsl],
                                qts[0:64, h, t * QT + off:(t + 1) * QT],
                                start=True, stop=True)
                        a2 = asb.tile([128, 2, QT], BF16, tag="a2", bufs=6)
                        nc.scalar.activation(a2[:, :, off:], s2[:, :, off:],
                                             AF.Exp)
                        if off or jc == 4 * t:  # diagonal block: mask 128 cols
                            nc.vector.tensor_mul(
                                a2[:, :, off:off + 128],
                                a2[:, :, off:off + 128], mask2[:, :, :])
                        for hh, h in ((0, h0), (1, h1)):
                            nc.tensor.matmul(
                                o_ps[hh][:, off:], vt[:, jc, h, :],
                                a2[:, hh, off:],
                                start=(jc == 0), stop=(jc == njc - 1),
                                skip_group_check=True)
                    # normalize: inv = 1/sums (ones-row of o_ps), broadcast,
                    # multiply into ot (bf16, ready for the out-projection)
                    bci = st.tile([64, 2, QT], F32, tag="bci", bufs=2)
                    for hh, h in ((0, h0), (1, h1)):
                        nc.vector.reciprocal(inv[h][0:1, isl],
                                             o_ps[hh][64:65, :])
                        nc.gpsimd.partition_broadcast(
                            bci[:, hh, :], inv[h][0:1, isl])
                    for hh in range(2):
                        nc.vector.tensor_mul(
                            ot[64 * hh:64 * hh + 64, p, isl],
                            o_ps[hh][0:64, :], bci[:, hh, :])

            def outproj_tile(t):
                tsl = slice(t * QT, (t + 1) * QT)
                for mc in range(MOUT // 128):
                    ops = psM.tile([128, QT], F32, tag="mm_ps")
                    for dc in range(2):
                        nc.tensor.matmul(
                            ops[:, :],
                            wo_sb[:, dc, mc * 128:(mc + 1) * 128],
                            ot[:, dc, tsl],
                            start=(dc == 0), stop=(dc == 1))
                    osb = st.tile([128, QT], BF16, tag="osb", bufs=3)
                    if mc % 2 == 0:
                        nc.scalar.copy(osb[:, :], ops[:, :])
                    else:
                        nc.vector.tensor_copy(osb[:, :], ops[:, :])
                    nc.sync.dma_start(out_ext[mc * 128:(mc + 1) * 128, tsl],
                                      osb[:, :])

            for t in range(NQT):
                qkv_tile(t)
                if t > 0:
                    outproj_tile(t - 1)
                attn_tile(t)
            outproj_tile(NQT - 1)

    nc.compile()
    return nc


_NC_CACHE = {}


def _get_nc(N=N_TOK):
    if N not in _NC_CACHE:
        _NC_CACHE[N] = build_nc(N)
    return _NC_CACHE[N]


def _marshal_w(w):
    """[c*128, n] -> [128, c, n] bf16 contiguous (device SBUF layout)."""
    c = w.shape[0] // 128
    return np.ascontiguousarray(
        w.reshape(c, 128, -1).transpose(1, 0, 2)).astype(mybir.dt.np(BF16))


def make_in_maps(x, Wq, Wk, Wv, Wo):
    in_maps = []
    for c in range(8):
        b, g = divmod(c, 2)
        gsl = slice(g * DG, (g + 1) * DG)
        in_maps.append({
            "xt": np.ascontiguousarray(x[b].T).astype(mybir.dt.np(BF16)),
            "wq": _marshal_w(Wq[:, gsl]),
            "wk": _marshal_w(Wk[:, gsl]),
            "wv": _marshal_w(Wv[:, gsl]),
            "wo": _marshal_w(Wo[gsl, :]),
        })
    return in_maps


def kernel(x, Wq, Wk, Wv, Wo, _trace=False):
    x = np.asarray(x)
    nc = _get_nc(x.shape[1])
    in_maps = make_in_maps(np.asarray(x), np.asarray(Wq), np.asarray(Wk),
                           np.asarray(Wv), np.asarray(Wo))
    res = run_bass_kernel_spmd(nc, in_maps, core_ids=list(range(8)),
                               trace=_trace)
    kernel.last_results = res
    out = np.empty((x.shape[0], x.shape[1], MOUT), dtype=np.float32)
    for b in range(x.shape[0]):
        a = res.results[2 * b]["out"].astype(np.float32)
        c = res.results[2 * b + 1]["out"].astype(np.float32)
        out[b] = (a + c).T
    return out
